# revision 39
# baseline (speedup 1.0000x reference)
"""AdaLN attention block (DiT-style) on 8 TRN2 NeuronCores.

Sharding: 8 cores = 4 batches x 2 token-halves, no collectives. Core c handles
batch c//2 and query-token half c%2: layernorm1 and k/v are computed over the
full (permuted) sequence, everything else only for the own 512 query rows.

Device layout is feature-major (activations transposed, [d, n]). X @ W runs
with W column-tiles stationary and X^T moving, producing Y^T directly.
LayerNorm statistics use ones-vector matmuls (partition-axis sums on the PE);
the AdaLN modulate is h = x*A + B with rank-1 A/B built by K=1 outer-product
matmuls into PSUM. Softmax skips max-subtraction (fp32 exp is safe for this
distribution); the denominator is a ones-column appended to the attn@v
stationary operand.

v2 changes vs baseline:
- istds via exp(-0.5*ln(var+eps)) on ACT; softmax 1/denom via
  reciprocal_approx_fast (custom DVE); silu via exp+recip. All ACT
  transcendentals in the ln+exp table set, pinned by post-processing the
  act-table-load pass (avoids per-call table thrash).
- exp batched over [128,1024] merged sim PSUM (both heads of a pair).
- Wo projection folded into the attention loop as per-head-pair partial
  accumulation into SBUF; bias/bo prep moved to the qkv phase.
- k/q PSUM evictions moved from ACT to DVE tensor_scalar.
- denominator broadcast via gpsimd partition_broadcast (no PSUM/PE).
- DMA issue order prioritizes the critical path: c, x, Wada[g0:4], Wq,
  Wkv, Wo, xo, Wada[g4:], W1, W2. mod groups 4-11 are PE filler inside
  the attention loop.
"""

import numpy as np
from contextlib import ExitStack

import concourse.bass as bass
import concourse.bacc as bacc
import concourse.mybir as mybir
from concourse import tile
from concourse.bass_utils import run_bass_kernel_spmd

P = 128
D = 1024
N = 1024
NQ = 512
H = 16
DH = 64
MLPD = 4096
EPS = 1e-6
NCORES = 8

F32 = mybir.dt.float32
BF16 = mybir.dt.bfloat16
AF = mybir.ActivationFunctionType
ALU = mybir.AluOpType

KT = D // P           # 8 contraction tiles over D
MT = MLPD // P        # 32 tiles over MLP dim

LN_EXP_SET = "natural_log_exp_and_others"


def _r(ap):
    return ap


def _pin_act_tables(nc):
    """Rewrite act-table loads so every set containing only {ln, exp,
    identity/copy/...-compatible} functions maps to the shared ln+exp set,
    then drop loads that are redundant in straight-line order."""
    from concourse.hw_specs import get_activation_tables
    tables = list(get_activation_tables(nc.m.arch).keys())
    target = tables.index(LN_EXP_SET)
    # sets whose anchor functions are covered by the ln+exp set
    remap = set()
    for i, name in enumerate(tables):
        if name in ("exp_and_others", "natural_log", LN_EXP_SET):
            remap.add(i)
    for blk in nc.main_func.blocks:
        cur = None
        dead = []
        for inst in blk.instructions:
            if type(inst).__name__.endswith("InstLoadActFuncSet") or \
                    isinstance(inst, getattr(mybir, "InstLoadActFuncSet", ())):
                sid = inst.act_func_set_id
                if sid in remap:
                    inst.act_func_set_id = target
                    sid = target
                if sid == cur:
                    dead.append(inst)
                else:
                    cur = sid
        for inst in dead:
            blk.instructions.remove(inst)


def build():
    nc = bacc.Bacc("TRN2", target_bir_lowering=False, debug=False,
                   num_devices=NCORES)

    xT = nc.dram_tensor("xT", [D, N], F32, kind="ExternalInput")
    crow = nc.dram_tensor("crow", [1, D], F32, kind="ExternalInput")
    Wq = nc.dram_tensor("Wq", [D, D], BF16, kind="ExternalInput")
    Wkv = nc.dram_tensor("Wkv", [D, 2 * D], BF16, kind="ExternalInput")
    Wo = nc.dram_tensor("Wo", [D, D], BF16, kind="ExternalInput")
    W1 = nc.dram_tensor("W1", [D, MLPD], BF16, kind="ExternalInput")
    W2 = nc.dram_tensor("W2", [MLPD, D], BF16, kind="ExternalInput")
    Wada = nc.dram_tensor("Wada", [D, 6 * D], BF16, kind="ExternalInput")
    bada_r = nc.dram_tensor("bada_r", [1, 6 * D], F32, kind="ExternalInput")
    bq_c = nc.dram_tensor("bq_c", [P, KT], F32, kind="ExternalInput")
    bk_c = nc.dram_tensor("bk_c", [P, KT], F32, kind="ExternalInput")
    bv_c = nc.dram_tensor("bv_c", [P, KT], BF16, kind="ExternalInput")
    bo_r = nc.dram_tensor("bo_r", [1, D], F32, kind="ExternalInput")
    b1_c = nc.dram_tensor("b1_c", [P, MT], F32, kind="ExternalInput")
    b2_c = nc.dram_tensor("b2_c", [P, KT], F32, kind="ExternalInput")
    yT = nc.dram_tensor("yT", [D, NQ], F32, kind="ExternalOutput")

    with tile.TileContext(nc) as tc, ExitStack() as root:
        const = root.enter_context(tc.tile_pool(name="const", bufs=1))
        rootrows = root.enter_context(tc.tile_pool(name="rootrows", bufs=1))

        # persistent pools (created early to sit at the bottom of the
        # side stacks; root-lifetime unless popped explicitly below)
        badp = root.enter_context(tc.tile_pool(name="badp", bufs=2))
        op_ = root.enter_context(tc.tile_pool(name="op", bufs=1,
                                              side='left'))
        outT = [op_.tile([P, NQ], BF16, tag=f"o{k}", name=f"o{k}")
                for k in range(KT)]
        hT_cm = tc.tile_pool(name="hTp", bufs=1, side='left')
        hTp = hT_cm.__enter__()
        hT = [hTp.tile([P, N], BF16, tag=f"h{k}", name=f"h{k}")
              for k in range(KT)]
        xop = root.enter_context(tc.tile_pool(name="xop", bufs=1,
                                              side='right'))
        xo = [xop.tile([P, NQ], F32, tag=f"xo{k}", name=f"xo{k}")
              for k in range(KT)]
        wop = root.enter_context(tc.tile_pool(name="wop", bufs=1,
                                              side='right'))
        woT = [wop.tile([P, D], BF16, tag=f"wo{k}", name=f"wo{k}")
               for k in range(KT)]

        # ---- critical-path DMAs first: c, then x tiles ----
        c_sb = rootrows.tile([1, D], F32, name='c_sb')
        nc.sync.dma_start(c_sb[:], crow[:])

        xp_cm = tc.tile_pool(name="xp", bufs=1, side='left')
        xpool = xp_cm.__enter__()
        xt = [xpool.tile([P, N], F32, tag=f"x{k}", name=f"x{k}")
              for k in range(KT)]
        for k in range(KT):
            eng = nc.sync if k % 2 == 0 else nc.gpsimd
            eng.dma_start(xt[k][:, 0:NQ], xT[k * P:(k + 1) * P, 0:NQ])
            eng2 = nc.gpsimd if k % 2 == 0 else nc.sync
            eng2.dma_start(xt[k][:, NQ:N], xT[k * P:(k + 1) * P, NQ:N])


        # Wada groups 0-3 (sh_msa, sc_msa) — on the critical path to h
        wada_cm = tc.tile_pool(name="wadap", bufs=16, side='left')
        wadap = wada_cm.__enter__()
        wada_early = {}
        for g in range(4):
            wada_early[g] = [wadap.tile([P, NQ], BF16, tag="wae",
                                        name='wae') for _ in range(KT)]
            for k in range(KT):
                eng = nc.sync if k % 2 == 0 else nc.gpsimd
                eng.dma_start(wada_early[g][k][:],
                              Wada[k * P:(k + 1) * P, g * NQ:(g + 1) * NQ])

        ones_col = const.tile([P, 1], BF16, name='ones_col')
        nc.vector.memset(ones_col[:], 1.0)
        ones_col_f = const.tile([P, 1], F32, name='ones_col_f')
        nc.vector.memset(ones_col_f[:], 1.0)
        ones_row = const.tile([1, NQ], BF16, name='ones_row')
        nc.vector.memset(ones_row[:], 1.0)
        eps_t = const.tile([1, 1], F32, name='eps_t')
        nc.vector.memset(eps_t[:], EPS)

        bqT = const.tile([P, KT], F32, name='bqT')
        nc.sync.dma_start(bqT[:], bq_c[:])
        bkT = const.tile([P, KT], F32, name='bkT')
        nc.sync.dma_start(bkT[:], bk_c[:])
        bvT = const.tile([P, KT], BF16, name='bvT')
        nc.sync.dma_start(bvT[:], bv_c[:])
        b1T = const.tile([P, MT], F32, name='b1T')
        nc.gpsimd.dma_start(b1T[:], b1_c[:])
        b2T = const.tile([P, KT], F32, name='b2T')
        nc.gpsimd.dma_start(b2T[:], b2_c[:])
        bo_row = const.tile([1, D], F32, name='bo_row')
        nc.gpsimd.dma_start(bo_row[:], bo_r[:])

        def cols_from_row(row_ap, dst, psum_pool, tag="colps"):
            """[1, n*128] row -> [128, n] column tile via K=1 matmuls."""
            n = dst.shape[-1]
            ps = psum_pool.tile([P, n], F32, tag=tag, name='colps')
            for j in range(n):
                nc.tensor.matmul(ps[:, j:j + 1],
                                 lhsT=_r(row_ap[0:1, j * P:(j + 1) * P]),
                                 rhs=_r(ones_row[0:1, 0:1]),
                                 start=True, stop=True)
            nc.vector.tensor_copy(dst[:], ps[:])
            return dst

        boT = const.tile([P, KT], F32, name='boT')
        csT = const.tile([P, KT], BF16, name='csT')
        modc = const.tile([P, 48], F32, name='modc')
        S1c = const.tile([P, KT], F32, name='S1c')
        S2c = const.tile([P, KT], F32, name='S2c')

        mod_row = rootrows.tile([1, 6 * D], BF16, name='mod_row')
        bop_row = rootrows.tile([1, D], BF16, name='bop_row')

        # ---------------- phase 0: silu(c), mod groups 0-3, ln1 ----------
        with ExitStack() as ph:
            rows = ph.enter_context(tc.tile_pool(name="p0rows", bufs=1))
            sqpool = ph.enter_context(tc.tile_pool(name="p0sq", bufs=4))

            with ExitStack() as sec:
                pscol = sec.enter_context(
                    tc.tile_pool(name="pscol", bufs=1, space="PSUM"))
                psmod = sec.enter_context(
                    tc.tile_pool(name="psmod", bufs=3, space="PSUM"))
                psstat = sec.enter_context(
                    tc.tile_pool(name="psstat", bufs=2, space="PSUM"))

                # silu(c) = c / (1 + exp(-c)) without the silu table set
                cexp = rows.tile([1, D], F32, name='cexp')
                nc.scalar.activation(cexp[:], c_sb[:], AF.Exp, scale=-1.0)
                nc.vector.tensor_scalar_add(cexp[:], cexp[:], 1.0)
                cinv = rows.tile([1, D], F32, name='cinv')
                nc.vector.reciprocal_approx_fast(cinv[:], cexp[:])
                cs_row = rows.tile([1, D], BF16, name='cs_row')
                nc.vector.tensor_mul(cs_row[:], c_sb[:], cinv[:])
                cols_from_row(cs_row, csT, pscol)

                # mod = silu(c) @ Wada + bada, one [1,512] group at a time
                def emit_mod_group(g, wch, psp, tag="modps"):
                    mp = psp.tile([1, NQ], F32, tag=tag, name='modps')
                    for k in range(KT):
                        nc.tensor.matmul(mp[:], lhsT=_r(csT[:, k:k + 1]),
                                         rhs=_r(wch[k][:]),
                                         start=(k == 0), stop=(k == KT - 1))
                    bad = badp.tile([1, NQ], F32, tag="bad", name='bad')
                    nc.sync.dma_start(bad[:],
                                      bada_r[0:1, g * NQ:(g + 1) * NQ])
                    nc.vector.tensor_add(
                        mod_row[0:1, g * NQ:(g + 1) * NQ], mp[:], bad[:])

                for g in range(4):
                    emit_mod_group(g, wada_early[g], psmod)
                # column layout for sh_msa / sc_msa regions
                cols_from_row(mod_row[0:1, 0:D], modc[:, 0:8], pscol)
                cols_from_row(mod_row[0:1, D:2 * D], modc[:, 8:16], pscol)

                # ln1 stats: per 512-chunk, sum and sumsq over d
                mu_row = rows.tile([1, N], F32, name='mu_row')
                ex2_row = rows.tile([1, N], F32, name='ex2_row')
                for ch in range(2):
                    sl = slice(ch * NQ, (ch + 1) * NQ)
                    ss = psstat.tile([1, NQ], F32, tag="st_s", name='st_s')
                    sq_ps = psstat.tile([1, NQ], F32, tag="st_q",
                                        name='st_q')
                    for k in range(KT):
                        sq = sqpool.tile([P, NQ], BF16, tag="xsq",
                                         name='xsq')
                        nc.scalar.activation(sq[:], xt[k][:, sl], AF.Square)
                        nc.tensor.matmul(ss[:], lhsT=_r(ones_col_f[:]),
                                         rhs=_r(xt[k][:, sl]),
                                         start=(k == 0), stop=(k == KT - 1))
                        nc.tensor.matmul(sq_ps[:], lhsT=_r(ones_col[:]),
                                         rhs=_r(sq[:]),
                                         start=(k == 0), stop=(k == KT - 1))
                    nc.vector.tensor_scalar_mul(mu_row[0:1, sl], ss[:],
                                                1.0 / D)
                    nc.vector.tensor_scalar_mul(ex2_row[0:1, sl], sq_ps[:],
                                                1.0 / D)

                var_row = rows.tile([1, N], F32, name='var_row')
                nc.vector.tensor_mul(var_row[:], mu_row[:], mu_row[:])
                nc.vector.tensor_sub(var_row[:], ex2_row[:], var_row[:])
                # istd = exp(-0.5 * ln(var + eps))
                lnv = rows.tile([1, N], F32, name='lnv')
                nc.scalar.activation(lnv[:], var_row[:], AF.Ln,
                                     bias=eps_t[:])
                a_row = rows.tile([1, N], BF16, name='a_row')
                nc.scalar.activation(a_row[:], lnv[:], AF.Exp, scale=-0.5)
                b_row = rows.tile([1, N], BF16, name='b_row')
                nc.vector.tensor_mul(b_row[:], mu_row[:], a_row[:])
                nc.vector.tensor_scalar_mul(b_row[:], b_row[:], -1.0)

                # modulation columns: S1 = 1 + sc_msa, sh1 = modc[:, 0:8]
                nc.vector.tensor_scalar_add(S1c[:], modc[:, 8:16], 1.0)
                S1_row = rows.tile([1, D], BF16, name='S1_row')
                nc.vector.tensor_scalar_add(S1_row[:],
                                            mod_row[0:1, D:2 * D], 1.0)
                sh1_row = mod_row[0:1, 0:D]
                # broadcasts of the ln1 istd rows for the DVE modulate
                a_bc = rows.tile([P, N], BF16, name='a_bc')
                nc.gpsimd.partition_broadcast(a_bc[:], a_row[:])
                b_bc = rows.tile([P, N], BF16, name='b_bc')
                nc.gpsimd.partition_broadcast(b_bc[:], b_row[:])

            # h own half on PE (feeds q/sim first; keeps PE busy)...
            with ExitStack() as sec2:
                psab = sec2.enter_context(
                    tc.tile_pool(name="psab", bufs=2, space="PSUM"))
                for k in range(KT):
                    sl = slice(0, NQ)
                    pa = psab.tile([P, NQ], F32, tag="pA", name='pA')
                    pb = psab.tile([P, NQ], F32, tag="pB", name='pB')
                    nc.tensor.matmul(
                        pa[:], lhsT=_r(S1_row[0:1, k * P:(k + 1) * P]),
                        rhs=_r(a_row[0:1, sl]), start=True, stop=True)
                    nc.tensor.matmul(
                        pb[:], lhsT=_r(S1_row[0:1, k * P:(k + 1) * P]),
                        rhs=_r(b_row[0:1, sl]), start=True, stop=False)
                    nc.tensor.matmul(
                        pb[:], lhsT=_r(sh1_row[0:1, k * P:(k + 1) * P]),
                        rhs=_r(ones_row[:]), start=False, stop=True)
                    nc.vector.tensor_mul(hT[k][:, sl], xt[k][:, sl],
                                         pa[:])
                    nc.vector.tensor_add(hT[k][:, sl], hT[k][:, sl],
                                         pb[:])

            # ...and the far half on DVE (overlaps the q projection MMs)
            for k in range(KT):
                sl = slice(NQ, N)
                t1 = sqpool.tile([P, NQ], BF16, tag="t1", name='t1')
                nc.vector.tensor_mul(t1[:], xt[k][:, sl], a_bc[:, sl])
                nc.vector.tensor_add(t1[:], t1[:], b_bc[:, sl])
                nc.vector.tensor_scalar(hT[k][:, sl], t1[:],
                                        S1c[:, k:k + 1],
                                        modc[:, k:k + 1],
                                        ALU.mult, ALU.add)

        wada_cm.__exit__(None, None, None)
        xp_cm.__exit__(None, None, None)

        # ---------------- phase 2: q, k, v projections + bo prep ----------
        qkv_cm = tc.tile_pool(name="qkvp", bufs=1, side='right')
        qkvp = qkv_cm.__enter__()
        qTt = [qkvp.tile([P, NQ], BF16, tag=f"q{k}", name=f"q{k}")
               for k in range(KT)]
        kTt = [qkvp.tile([P, N], BF16, tag=f"k{k}", name=f"k{k}")
               for k in range(KT)]
        vRt = [qkvp.tile([P, H * (DH + 1)], BF16, tag=f"v{k}", name=f"v{k}")
               for k in range(KT)]

        wada2_cm = tc.tile_pool(name="wada2", bufs=8, side='left')
        wada2p = wada2_cm.__enter__()

        # k-part weight chunks of Wkv (alive until last emit_kT in attention)
        wkv_cm = tc.tile_pool(name="wkvp", bufs=1, side='right')
        wkvp = wkv_cm.__enter__()
        wkc = {}
        for g in range(2):
            wkc[g] = [wkvp.tile([P, NQ], BF16, tag=f"kg{g}_{k}",
                                name=f"kg{g}_{k}") for k in range(KT)]
            for k in range(KT):
                eng = nc.sync if k % 2 == 0 else nc.gpsimd
                eng.dma_start(wkc[g][k][:],
                              Wkv[k * P:(k + 1) * P, g * NQ:(g + 1) * NQ])
        for vg in range(2):
            wkc['v', vg] = [wkvp.tile([P, NQ], BF16, tag=f"vg{vg}_{k}",
                                      name=f"vg{vg}_{k}")
                            for k in range(KT)]
            for k in range(KT):
                eng = nc.sync if k % 2 == 0 else nc.gpsimd
                eng.dma_start(wkc['v', vg][k][:],
                              Wkv[k * P:(k + 1) * P,
                                  D + vg * NQ:D + (vg + 1) * NQ])

        # full Wo resident (used in phase 4)
        for k in range(KT):
            eng = nc.sync if k % 2 == 0 else nc.gpsimd
            eng.dma_start(woT[k][:], Wo[k * P:(k + 1) * P, :])

        prj_cm = tc.tile_pool(name="prjps", bufs=1, space="PSUM",
                              side='right')
        prjps = prj_cm.__enter__()

        def emit_kT(t, ch):
            g, dot = t // 4, t % 4
            sl = slice(ch * NQ, (ch + 1) * NQ)
            p = prjps.tile([P, NQ], F32, tag="prj", name='prj')
            for k in range(KT):
                nc.tensor.matmul(
                    p[:], lhsT=_r(wkc[g][k][:, dot * P:(dot + 1) * P]),
                    rhs=_r(hT[k][:, sl]),
                    start=(k == 0), stop=(k == KT - 1))
            nc.vector.tensor_scalar_add(kTt[t][:, sl], p[:],
                                        bkT[:, t:t + 1])

        def emit_v(vg, nt):
            p = prjps.tile([P, NQ], F32, tag="prj", name='prj')
            for k in range(KT):
                nc.tensor.matmul(
                    p[:], lhsT=_r(hT[k][:, nt * P:(nt + 1) * P]),
                    rhs=_r(wkc['v', vg][k][:]),
                    start=(k == 0), stop=(k == KT - 1))
            vv = vRt[nt].rearrange("p (h w) -> p h w", w=DH + 1)
            pv = p.rearrange("p (h w) -> p h w", w=DH)
            nc.vector.tensor_copy(vv[:, vg * 8:(vg + 1) * 8, 0:DH], pv[:])

        with ExitStack() as ph:
            wpool = ph.enter_context(tc.tile_pool(name="p2w", bufs=18))
            ps = ph.enter_context(
                tc.tile_pool(name="p2ps", bufs=4, space="PSUM"))

            for nt in range(KT):
                vv = vRt[nt].rearrange("p (h w) -> p h w", w=DH + 1)
                nc.vector.memset(vv[:, :, DH:DH + 1], 1.0)

            def stationary_group(wdram, col0, movs, evict, tagp):
                wch = [wpool.tile([P, NQ], BF16, tag=tagp, name=tagp)
                       for _ in range(KT)]
                for k in range(KT):
                    eng = nc.sync if k % 2 == 0 else nc.gpsimd
                    eng.dma_start(
                        wch[k][:], wdram[k * P:(k + 1) * P, col0:col0 + NQ])
                for dot in range(4):
                    p = ps.tile([P, NQ], F32, tag="prj", name='prj')
                    for k in range(KT):
                        nc.tensor.matmul(
                            p[:], lhsT=_r(wch[k][:, dot * P:(dot + 1) * P]),
                            rhs=movs[k], start=(k == 0), stop=(k == KT - 1))
                    evict(dot, p)

            # q^T (own rows), scaled by 1/sqrt(DH) (bias added pre-scale)
            for g in range(2):
                def ev_q(dot, p, g=g):
                    t = 4 * g + dot
                    nc.vector.tensor_scalar(qTt[t][:], p[:],
                                            bqT[:, t:t + 1], DH ** -0.5,
                                            ALU.add, ALU.mult)
                stationary_group(Wq, g * NQ,
                                 [_r(hT[k][:, 0:NQ]) for k in range(KT)],
                                 ev_q, "wst")

            # bo' = bo + bv @ Wo  (columns for the folded Wo bias)
            with ExitStack() as sec:
                psv = sec.enter_context(
                    tc.tile_pool(name="psv", bufs=2, space="PSUM"))
                pscol2 = sec.enter_context(
                    tc.tile_pool(name="pscol2", bufs=1, space="PSUM"))
                for g in range(2):
                    mp = psv.tile([1, NQ], F32, tag="bvps", name='bvps')
                    for k in range(KT):
                        nc.tensor.matmul(
                            mp[:], lhsT=_r(bvT[:, k:k + 1]),
                            rhs=_r(woT[k][:, g * NQ:(g + 1) * NQ]),
                            start=(k == 0), stop=(k == KT - 1))
                    nc.vector.tensor_add(
                        bop_row[0:1, g * NQ:(g + 1) * NQ], mp[:],
                        bo_row[0:1, g * NQ:(g + 1) * NQ])
                    cols_from_row(bop_row[0:1, g * NQ:(g + 1) * NQ],
                                  boT[:, g * 4:(g + 1) * 4], pscol2)

            # k^T tiles 0-1 and v-group 0 now; the rest is emitted inside
            # the attention loop as just-in-time work that keeps PE warm
            for t in range(2):
                emit_kT(t, 0)
                emit_kT(t, 1)
            for nt in range(KT):
                emit_v(0, nt)

            # mod groups 4-11 fill the PE while DVE finishes the far-half h
            def emit_mod_late(g):
                wch = [wada2p.tile([P, NQ], BF16, tag="wal", name='wal')
                       for _ in range(KT)]
                for k in range(KT):
                    eng = nc.sync if k % 2 == 0 else nc.gpsimd
                    eng.dma_start(wch[k][:],
                                  Wada[k * P:(k + 1) * P,
                                       g * NQ:(g + 1) * NQ])
                emit_mod_group(g, wch, prjps, tag="prj")

            for g in range(4, 12):
                emit_mod_late(g)
            cols_from_row(mod_row[0:1, 2 * D:3 * D], modc[:, 16:24],
                          prjps, tag="prj")
            cols_from_row(mod_row[0:1, 5 * D:6 * D], modc[:, 40:48],
                          prjps, tag="prj")
            S2_row = rootrows.tile([1, D], BF16, name='S2_row')
            nc.vector.tensor_scalar_add(S2_row[:],
                                        mod_row[0:1, 4 * D:5 * D], 1.0)
            sh2_row_t = rootrows.tile([1, D], BF16, name='sh2_row_t')
            nc.vector.tensor_copy(sh2_row_t[:], mod_row[0:1, 3 * D:4 * D])

        # x own-half reload for the residual (needed in phase 4)
        for k in range(KT):
            eng = nc.sync if k % 2 == 0 else nc.gpsimd
            eng.dma_start(xo[k][:], xT[k * P:(k + 1) * P, 0:NQ])

        # ---------------- phase 3: attention ----------------
        with ExitStack() as ph:
            epool = ph.enter_context(tc.tile_pool(name="p3e", bufs=20))
            spool = ph.enter_context(tc.tile_pool(name="p3s", bufs=1))
            bpool = ph.enter_context(tc.tile_pool(name="p3b", bufs=1))
            ps_sim = ph.enter_context(
                tc.tile_pool(name="ps_sim", bufs=3, space="PSUM"))
            ps_o = ph.enter_context(
                tc.tile_pool(name="ps_o", bufs=2, space="PSUM"))

            for hp in range(H // 2):
                pt = hp
                # JIT emissions to keep PE warm while ACT runs exp
                if 0 < hp < 7:
                    emit_kT(hp + 1, 0)
                    emit_kT(hp + 1, 1)
                if hp < 4:
                    emit_v(1, 2 * hp)
                    emit_v(1, 2 * hp + 1)

                et = {0: [], 1: []}
                for kt in range(KT):
                    pp = {}
                    for hi in range(2):
                        hh = hi * DH
                        p = ps_sim.tile([P, NQ], F32, tag="sim",
                                        name='sim')
                        nc.tensor.matmul(
                            p[:],
                            lhsT=_r(kTt[pt][hh:hh + DH,
                                            kt * P:(kt + 1) * P]),
                            rhs=_r(qTt[pt][hh:hh + DH, :]),
                            start=True, stop=True)
                        pp[hi] = p
                    for hi in range(2):
                        e = epool.tile([P, NQ], BF16, tag="e", name='e')
                        nc.scalar.activation(e[:], pp[hi][:], AF.Exp)
                        et[hi].append(e)

                pos = ps_o.tile([DH + 1, 2 * NQ], F32, tag="ov", name='ov')
                for kt in range(KT):
                    for hi in range(2):
                        h = 2 * hp + hi
                        nc.tensor.matmul(
                            pos[:, hi * NQ:(hi + 1) * NQ],
                            lhsT=_r(vRt[kt][:, h * (DH + 1):
                                            (h + 1) * (DH + 1)]),
                            rhs=_r(et[hi][kt][:]),
                            start=(kt == 0), stop=(kt == KT - 1))

                # custom-DVE recip misreads PSUM on HW — bounce via SBUF
                drow = spool.tile([1, 2 * NQ], F32, tag="drow", name='drow')
                nc.vector.tensor_copy(drow[:], pos[DH:DH + 1, :])
                inv_row = spool.tile([1, 2 * NQ], F32, tag="invs",
                                     name='invs')
                nc.vector.reciprocal_approx_fast(inv_row[:], drow[:])
                binv = bpool.tile([P, 2 * NQ], F32, tag="binv",
                                  name='binv')
                nc.gpsimd.partition_broadcast(binv[:], inv_row[:])
                for hi in range(2):
                    hh = hi * DH
                    nc.vector.tensor_mul(
                        outT[pt][hh:hh + DH, :],
                        pos[0:DH, hi * NQ:(hi + 1) * NQ],
                        binv[hh:hh + DH, hi * NQ:(hi + 1) * NQ])

        prj_cm.__exit__(None, None, None)
        wkv_cm.__exit__(None, None, None)
        qkv_cm.__exit__(None, None, None)
        wada2_cm.__exit__(None, None, None)
        hT_cm.__exit__(None, None, None)

        # ---------------- phase 4: Wo + residual + ln2 + modulate ----------
        x1p = root.enter_context(tc.tile_pool(name="x1p", bufs=1,
                                              side='right'))
        x1t = [x1p.tile([P, NQ], F32, tag=f"x1{k}", name=f"x1{k}")
               for k in range(KT)]
        h2p = root.enter_context(tc.tile_pool(name="h2p", bufs=1,
                                              side='right'))
        h2t = [h2p.tile([P, NQ], BF16, tag=f"h2{k}", name=f"h2{k}")
               for k in range(KT)]

        with ExitStack() as ph:
            rows4 = ph.enter_context(tc.tile_pool(name="p4rows", bufs=1))
            tpool = ph.enter_context(tc.tile_pool(name="p4t", bufs=3))

            with ExitStack() as sec:
                psy = sec.enter_context(
                    tc.tile_pool(name="psy", bufs=2, space="PSUM"))
                psstat2 = sec.enter_context(
                    tc.tile_pool(name="psstat2", bufs=1, space="PSUM"))
                psab2 = sec.enter_context(
                    tc.tile_pool(name="psab2", bufs=2, space="PSUM"))

                ss = psstat2.tile([1, NQ], F32, tag="st2s", name='st2s')
                sq_ps = psstat2.tile([1, NQ], F32, tag="st2q", name='st2q')
                for k in range(KT):
                    p = psy.tile([P, NQ], F32, tag="y1", name='y1')
                    for j in range(KT):
                        nc.tensor.matmul(
                            p[:], lhsT=_r(woT[j][:, k * P:(k + 1) * P]),
                            rhs=_r(outT[j][:]),
                            start=(j == 0), stop=(j == KT - 1))
                    tmp = tpool.tile([P, NQ], F32, tag="y1s", name='y1s')
                    nc.vector.tensor_scalar(tmp[:], p[:],
                                            boT[:, k:k + 1],
                                            modc[:, 16 + k:17 + k],
                                            ALU.add, ALU.mult)
                    nc.vector.tensor_add(x1t[k][:], xo[k][:], tmp[:])
                    sq = tpool.tile([P, NQ], BF16, tag="x1sq", name='x1sq')
                    nc.scalar.activation(sq[:], x1t[k][:], AF.Square)
                    nc.tensor.matmul(ss[:], lhsT=_r(ones_col_f[:]),
                                     rhs=_r(x1t[k][:]),
                                     start=(k == 0), stop=(k == KT - 1))
                    nc.tensor.matmul(sq_ps[:], lhsT=_r(ones_col[:]),
                                     rhs=_r(sq[:]),
                                     start=(k == 0), stop=(k == KT - 1))
                mu2 = rows4.tile([1, NQ], F32, name='mu2')
                ex22 = rows4.tile([1, NQ], F32, name='ex22')
                nc.vector.tensor_scalar_mul(mu2[:], ss[:], 1.0 / D)
                nc.vector.tensor_scalar_mul(ex22[:], sq_ps[:], 1.0 / D)
                var2 = rows4.tile([1, NQ], F32, name='var2')
                nc.vector.tensor_mul(var2[:], mu2[:], mu2[:])
                nc.vector.tensor_sub(var2[:], ex22[:], var2[:])
                lnv2 = rows4.tile([1, NQ], F32, name='lnv2')
                nc.scalar.activation(lnv2[:], var2[:], AF.Ln, bias=eps_t[:])
                a2 = rows4.tile([1, NQ], BF16, name='a2')
                nc.scalar.activation(a2[:], lnv2[:], AF.Exp, scale=-0.5)
                b2r = rows4.tile([1, NQ], BF16, name='b2r')
                nc.vector.tensor_mul(b2r[:], mu2[:], a2[:])
                nc.vector.tensor_scalar_mul(b2r[:], b2r[:], -1.0)

                for k in range(KT):
                    pa = psab2.tile([P, NQ], F32, tag="pA2", name='pA2')
                    pb = psab2.tile([P, NQ], F32, tag="pB2", name='pB2')
                    nc.tensor.matmul(
                        pa[:], lhsT=_r(S2_row[0:1, k * P:(k + 1) * P]),
                        rhs=_r(a2[:]), start=True, stop=True)
                    nc.tensor.matmul(
                        pb[:], lhsT=_r(S2_row[0:1, k * P:(k + 1) * P]),
                        rhs=_r(b2r[:]), start=True, stop=False)
                    nc.tensor.matmul(
                        pb[:], lhsT=_r(sh2_row_t[0:1, k * P:(k + 1) * P]),
                        rhs=_r(ones_row[:]), start=False, stop=True)
                    nc.vector.tensor_mul(h2t[k][:], x1t[k][:], pa[:])
                    nc.vector.tensor_add(h2t[k][:], h2t[k][:], pb[:])

        # ---------------- phase 5: MLP ----------------
        with ExitStack() as ph:
            gp = ph.enter_context(tc.tile_pool(name="gp", bufs=1))
            gTt = [gp.tile([P, NQ], BF16, tag=f"g{m}", name=f"g{m}")
                   for m in range(MT)]
            wpool = ph.enter_context(tc.tile_pool(name="p5w", bufs=24))
            w2pool = ph.enter_context(tc.tile_pool(name="p5w2", bufs=16))
            opool = ph.enter_context(tc.tile_pool(name="p5o", bufs=3))
            ps1 = ph.enter_context(
                tc.tile_pool(name="ps1", bufs=4, space="PSUM"))
            ps2 = ph.enter_context(
                tc.tile_pool(name="ps2", bufs=1, space="PSUM"))

            for g in range(MLPD // NQ):   # 8 column groups
                wch = [wpool.tile([P, NQ], BF16, tag="w1", name='w1')
                       for _ in range(KT)]
                for k in range(KT):
                    eng = nc.sync if k % 2 == 0 else nc.gpsimd
                    eng.dma_start(
                        wch[k][:], W1[k * P:(k + 1) * P,
                                      g * NQ:(g + 1) * NQ])
                for dot in range(4):
                    m = 4 * g + dot
                    p = ps1.tile([P, NQ], F32, tag="m1", name='m1')
                    for k in range(KT):
                        nc.tensor.matmul(
                            p[:], lhsT=_r(wch[k][:, dot * P:(dot + 1) * P]),
                            rhs=_r(h2t[k][:]),
                            start=(k == 0), stop=(k == KT - 1))
                    nc.scalar.activation(gTt[m][:], p[:], AF.Gelu_apprx_tanh,
                                         bias=b1T[:, m:m + 1])

            for half in range(2):
                pacc = [ps2.tile([P, NQ], F32, tag=f"acc{d}",
                                 name=f"acc{d}") for d in range(4)]
                for mk in range(MT):
                    w2c = w2pool.tile([P, NQ], BF16, tag="w2", name='w2')
                    eng = nc.sync if mk % 2 == 0 else nc.gpsimd
                    eng.dma_start(
                        w2c[:], W2[mk * P:(mk + 1) * P,
                                   half * NQ:(half + 1) * NQ])
                    for d in range(4):
                        nc.tensor.matmul(
                            pacc[d][:],
                            lhsT=_r(w2c[:, d * P:(d + 1) * P]),
                            rhs=_r(gTt[mk][:]),
                            start=(mk == 0), stop=(mk == MT - 1))
                for d in range(4):
                    t = half * 4 + d
                    tmp = opool.tile([P, NQ], F32, tag="m2s", name='m2s')
                    nc.vector.tensor_scalar(tmp[:], pacc[d][:],
                                            b2T[:, t:t + 1],
                                            modc[:, 40 + t:41 + t],
                                            ALU.add, ALU.mult)
                    yt = opool.tile([P, NQ], F32, tag="yout", name='yout')
                    nc.vector.tensor_add(yt[:], x1t[t][:], tmp[:])
                    eng = nc.sync if t % 2 == 0 else nc.gpsimd
                    eng.dma_start(yT[t * P:(t + 1) * P, :], yt[:])

    orig_atl = nc.insert_act_table_loads

    def patched_atl():
        orig_atl()
        _pin_act_tables(nc)

    nc.insert_act_table_loads = patched_atl
    nc.compile()
    return nc


_NC = None


def _get_nc():
    global _NC
    if _NC is None:
        _NC = build()
    return _NC


def _prep_inputs(x, c, Wq, bq, Wkv, bkv, Wo, bo, W1, b1, W2, b2, Wada, bada):
    import ml_dtypes
    f = np.float32
    bf = ml_dtypes.bfloat16
    col = lambda v, n: np.ascontiguousarray(
        np.asarray(v, f).reshape(n, P).T)
    shared = {
        "Wq": np.asarray(Wq, f).astype(bf), "Wkv": np.asarray(Wkv, f).astype(bf),
        "Wo": np.asarray(Wo, f).astype(bf), "W1": np.asarray(W1, f).astype(bf),
        "W2": np.asarray(W2, f).astype(bf), "Wada": np.asarray(Wada, f).astype(bf),
        "bada_r": np.asarray(bada, f).reshape(1, -1),
        "bq_c": col(bq, KT), "bk_c": col(np.asarray(bkv, f)[:D], KT),
        "bv_c": col(np.asarray(bkv, f)[D:], KT).astype(bf),
        "bo_r": np.asarray(bo, f).reshape(1, -1),
        "b1_c": col(b1, MT), "b2_c": col(b2, KT),
    }
    in_maps = []
    for core in range(NCORES):
        b, half = core // 2, core % 2
        xb = np.asarray(x[b], f)
        perm = np.concatenate(
            [xb[half * NQ:(half + 1) * NQ],
             xb[(1 - half) * NQ:(2 - half) * NQ]], axis=0)
        m = dict(shared)
        m["xT"] = np.ascontiguousarray(perm.T)
        m["crow"] = np.asarray(c[b:b + 1], f)
        in_maps.append(m)
    return in_maps


def _run(inputs, trace=False):
    nc = _get_nc()
    in_maps = _prep_inputs(**inputs)
    res = run_bass_kernel_spmd(nc, in_maps, core_ids=list(range(NCORES)),
                               trace=trace)
    B = 4
    y = np.empty((B, N, D), np.float32)
    for core in range(NCORES):
        b, half = core // 2, core % 2
        y[b, half * NQ:(half + 1) * NQ, :] = res.results[core]["yT"].T
    return y, res


def kernel(**inputs):
    y, _ = _run(inputs, trace=False)
    return y
